# revision 1
# baseline (speedup 1.0000x reference)
"""Distributed ActorHyperRNN kernel for 8 Trainium2 NeuronCores.

Sharding strategy (per the problem's sharding hint):
  - Data-parallel over the batch axis B=128 -> 16 batch elements per core.
  - All parameters are replicated (they are small).
  - The GRU recurrence is only over time, so it shards cleanly over B.
  - The transformer / hypernet parts are parallel over T*B samples and are
    computed on each core for its own B-shard (all T).

kernel(**inputs) takes FULL (unsharded) numpy inputs, shards them internally,
runs SPMD across the 8 NeuronCores, and gathers the FULL output:
  (hidden [B, FC] float32, logits [T, B, A_DIM] float32)
matching reference.reference(**inputs).
"""

import numpy as np
import jax
import jax.numpy as jnp

# Model dims (hardcoded; kernel.py must be self-contained).
T, B, A_DIM = 64, 128, 5
N_AGENTS, N_CAP = 8, 7
OBS_D = 64
CAP_D = N_AGENTS * N_CAP  # 56
FC, HN, TGT = 128, 128, 64
D = N_CAP + 1  # 8
NH, FF, NL = 4, 64, 2
EPS = 1e-6

N_CORES = 8
BC = B // N_CORES  # 16 batch elements per core

_PARAM_NAMES = (
    "embed_w", "embed_b", "gru_wi", "gru_wh", "gru_bh",
    "tWq", "tWk", "tWv", "tWo", "ln1_s", "ln1_b",
    "ffw1", "ffb1", "ffw2", "ffb2", "ln2_s", "ln2_b",
    "hw1_1", "hb1_1", "hw2_1", "hb2_1",
    "hw1_2", "hb1_2", "hw2_2", "hb2_2",
    "hw1_3", "hb1_3", "hw2_3", "hb2_3",
    "hw1_4", "hb1_4", "hw2_4", "hb2_4",
)


def _ln(x, s, b):
    m = jnp.mean(x, -1, keepdims=True)
    v = jnp.mean(jnp.square(x - m), -1, keepdims=True)
    return (x - m) * jax.lax.rsqrt(v + EPS) * s + b


def _hyper(x, w1, b1, w2, b2):
    return jax.nn.relu(x @ w1 + b1) @ w2 + b2


def _shard_fn(hidden, full_obs, dones, *params):
    """Per-core computation on one B-shard.

    hidden: [BC, FC], full_obs: [T, BC, OBS_D+CAP_D], dones: [T, BC]
    """
    (embed_w, embed_b, gru_wi, gru_wh, gru_bh,
     tWq, tWk, tWv, tWo, ln1_s, ln1_b,
     ffw1, ffb1, ffw2, ffb2, ln2_s, ln2_b,
     hw1_1, hb1_1, hw2_1, hb2_1,
     hw1_2, hb1_2, hw2_2, hb2_2,
     hw1_3, hb1_3, hw2_3, hb2_3,
     hw1_4, hb1_4, hw2_4, hb2_4) = params

    obs = full_obs[:, :, :OBS_D]
    cap = full_obs[:, :, OBS_D:]
    emb = jax.nn.relu(obs @ embed_w + embed_b)  # [T, BC, FC]
    resets = dones.astype(bool)

    def gru_step(h, xt):
        e, d = xt
        h = jnp.where(d[:, None], 0.0, h)
        gi = e @ gru_wi
        gh = h @ gru_wh + gru_bh
        ir, iz, inn = jnp.split(gi, 3, -1)
        hr, hz, hn = jnp.split(gh, 3, -1)
        r = jax.nn.sigmoid(ir + hr)
        z = jax.nn.sigmoid(iz + hz)
        n = jnp.tanh(inn + r * hn)
        hnew = (1.0 - z) * n + z * h
        return hnew, hnew

    hidden, emb_seq = jax.lax.scan(gru_step, hidden, (emb, resets))

    # Transformer over the agents dim, for all T*BC samples of this shard.
    n_samp = T * BC
    x = cap.reshape(n_samp, N_AGENTS, N_CAP)
    flag = jnp.zeros((n_samp, N_AGENTS, 1), jnp.float32).at[:, 0, 0].set(1.0)
    x = jnp.concatenate([x, flag], axis=-1)  # [N, A, D]
    hd = D // NH
    scale = 1.0 / jnp.sqrt(jnp.float32(hd))
    for l in range(NL):
        q = (x @ tWq[l]).reshape(-1, N_AGENTS, NH, hd)
        k = (x @ tWk[l]).reshape(-1, N_AGENTS, NH, hd)
        v = (x @ tWv[l]).reshape(-1, N_AGENTS, NH, hd)
        att = jax.nn.softmax(jnp.einsum("nqhd,nkhd->nhqk", q, k) * scale, axis=-1)
        o = jnp.einsum("nhqk,nkhd->nqhd", att, v).reshape(-1, N_AGENTS, D) @ tWo[l]
        x = 2.0 * _ln(o + x, ln1_s[l], ln1_b[l])
        ff = jax.nn.relu(x @ ffw1[l] + ffb1[l]) @ ffw2[l] + ffb2[l]
        x = 2.0 * _ln(ff + x, ln2_s[l], ln2_b[l])
    cap_repr = jnp.mean(x, axis=1).reshape(T, BC, D)

    w1 = _hyper(cap_repr, hw1_1, hb1_1, hw2_1, hb2_1).reshape(T, BC, FC, TGT)
    b1 = _hyper(cap_repr, hw1_2, hb1_2, hw2_2, hb2_2).reshape(T, BC, 1, TGT)
    w2 = _hyper(cap_repr, hw1_3, hb1_3, hw2_3, hb2_3).reshape(T, BC, TGT, A_DIM)
    b2 = _hyper(cap_repr, hw1_4, hb1_4, hw2_4, hb2_4).reshape(T, BC, 1, A_DIM)

    h1 = jax.nn.relu(emb_seq[:, :, None, :] @ w1 + b1)
    logits = (h1 @ w2 + b2).squeeze(2)  # [T, BC, A_DIM]
    return hidden, logits


_pmapped = None


def _get_pmapped():
    global _pmapped
    if _pmapped is None:
        in_axes = (0, 0, 0) + (None,) * len(_PARAM_NAMES)
        _pmapped = jax.pmap(
            _shard_fn,
            in_axes=in_axes,
            devices=jax.devices()[:N_CORES],
        )
    return _pmapped


def kernel(**inputs):
    # Keep full fp32 matmul precision on the PE (default would downcast to
    # bf16-ish precision; the hypernet contractions need f32 accuracy).
    with jax.default_matmul_precision("highest"):
        hidden = jnp.asarray(inputs["hidden"], jnp.float32)
        full_obs = jnp.asarray(inputs["full_obs"], jnp.float32)
        dones = jnp.asarray(inputs["dones"], jnp.int32)
        params = tuple(jnp.asarray(inputs[k], jnp.float32) for k in _PARAM_NAMES)

        # Shard over B: core c takes b in [c*BC, (c+1)*BC).
        hidden_sh = hidden.reshape(N_CORES, BC, FC)
        obs_sh = full_obs.reshape(T, N_CORES, BC, OBS_D + CAP_D).transpose(1, 0, 2, 3)
        dones_sh = dones.reshape(T, N_CORES, BC).transpose(1, 0, 2)

        fn = _get_pmapped()
        hidden_out, logits_out = fn(hidden_sh, obs_sh, dones_sh, *params)

        # Gather / unshard to full shapes.
        hidden_full = np.asarray(hidden_out).reshape(B, FC).astype(np.float32)
        logits_full = (
            np.asarray(logits_out).transpose(1, 0, 2, 3).reshape(T, B, A_DIM)
        ).astype(np.float32)
    return hidden_full, logits_full


if __name__ == "__main__":
    # Smoke test with random inputs of the right shapes.
    rng = np.random.default_rng(0)
    inputs = dict(
        hidden=np.zeros((B, FC), np.float32),
        full_obs=rng.standard_normal((T, B, OBS_D + CAP_D), dtype=np.float32),
        dones=rng.integers(0, 2, (T, B)).astype(np.int32),
        embed_w=rng.standard_normal((OBS_D, FC), dtype=np.float32) * 0.1,
        embed_b=np.zeros((FC,), np.float32),
        gru_wi=rng.standard_normal((FC, 3 * FC), dtype=np.float32) * 0.1,
        gru_wh=rng.standard_normal((FC, 3 * FC), dtype=np.float32) * 0.1,
        gru_bh=np.zeros((3 * FC,), np.float32),
        tWq=rng.standard_normal((NL, D, D), dtype=np.float32) * 0.3,
        tWk=rng.standard_normal((NL, D, D), dtype=np.float32) * 0.3,
        tWv=rng.standard_normal((NL, D, D), dtype=np.float32) * 0.3,
        tWo=rng.standard_normal((NL, D, D), dtype=np.float32) * 0.3,
        ln1_s=np.ones((NL, D), np.float32), ln1_b=np.zeros((NL, D), np.float32),
        ffw1=rng.standard_normal((NL, D, FF), dtype=np.float32) * 0.2,
        ffb1=np.zeros((NL, FF), np.float32),
        ffw2=rng.standard_normal((NL, FF, D), dtype=np.float32) * 0.2,
        ffb2=np.zeros((NL, D), np.float32),
        ln2_s=np.ones((NL, D), np.float32), ln2_b=np.zeros((NL, D), np.float32),
        hw1_1=rng.standard_normal((D, HN), dtype=np.float32) * 0.1,
        hb1_1=np.zeros((HN,), np.float32),
        hw2_1=rng.standard_normal((HN, FC * TGT), dtype=np.float32) * 0.02,
        hb2_1=np.zeros((FC * TGT,), np.float32),
        hw1_2=rng.standard_normal((D, HN), dtype=np.float32) * 0.1,
        hb1_2=np.zeros((HN,), np.float32),
        hw2_2=rng.standard_normal((HN, TGT), dtype=np.float32) * 0.02,
        hb2_2=np.zeros((TGT,), np.float32),
        hw1_3=rng.standard_normal((D, HN), dtype=np.float32) * 0.1,
        hb1_3=np.zeros((HN,), np.float32),
        hw2_3=rng.standard_normal((HN, TGT * A_DIM), dtype=np.float32) * 0.01,
        hb2_3=np.zeros((TGT * A_DIM,), np.float32),
        hw1_4=rng.standard_normal((D, HN), dtype=np.float32) * 0.1,
        hb1_4=np.zeros((HN,), np.float32),
        hw2_4=rng.standard_normal((HN, A_DIM), dtype=np.float32) * 0.01,
        hb2_4=np.zeros((A_DIM,), np.float32),
    )
    h, lg = kernel(**inputs)
    print("hidden:", h.shape, h.dtype, "logits:", lg.shape, lg.dtype)


# revision 3
# speedup vs baseline: 1.4386x; 1.4386x over previous
"""Distributed ActorHyperRNN kernel for 8 Trainium2 NeuronCores.

Sharding strategy (per the problem's sharding hint):
  - Data-parallel over the batch axis B=128 -> 16 batch elements per core.
  - All parameters are replicated (they are small).
  - The GRU recurrence is only over time, so it shards cleanly over B.
  - The transformer / hypernet parts are parallel over T*B samples and are
    computed on each core for its own B-shard (all T).

kernel(**inputs) takes FULL (unsharded) numpy inputs, shards them internally,
runs SPMD across the 8 NeuronCores, and gathers the FULL output:
  (hidden [B, FC] float32, logits [T, B, A_DIM] float32)
matching reference.reference(**inputs).
"""

import numpy as np
import jax
import jax.numpy as jnp

# Model dims (hardcoded; kernel.py must be self-contained).
T, B, A_DIM = 64, 128, 5
N_AGENTS, N_CAP = 8, 7
OBS_D = 64
CAP_D = N_AGENTS * N_CAP  # 56
FC, HN, TGT = 128, 128, 64
D = N_CAP + 1  # 8
NH, FF, NL = 4, 64, 2
EPS = 1e-6

N_CORES = 8
BC = B // N_CORES  # 16 batch elements per core

_PARAM_NAMES = (
    "embed_w", "embed_b", "gru_wi", "gru_wh", "gru_bh",
    "tWq", "tWk", "tWv", "tWo", "ln1_s", "ln1_b",
    "ffw1", "ffb1", "ffw2", "ffb2", "ln2_s", "ln2_b",
    "hw1_1", "hb1_1", "hw2_1", "hb2_1",
    "hw1_2", "hb1_2", "hw2_2", "hb2_2",
    "hw1_3", "hb1_3", "hw2_3", "hb2_3",
    "hw1_4", "hb1_4", "hw2_4", "hb2_4",
)


def _ln(x, s, b):
    m = jnp.mean(x, -1, keepdims=True)
    v = jnp.mean(jnp.square(x - m), -1, keepdims=True)
    return (x - m) * jax.lax.rsqrt(v + EPS) * s + b


def _hyper(x, w1, b1, w2, b2):
    return jax.nn.relu(x @ w1 + b1) @ w2 + b2


def _shard_fn(hidden, full_obs, dones, *params):
    """Per-core computation on one B-shard.

    hidden: [BC, FC], full_obs: [T, BC, OBS_D+CAP_D], dones: [T, BC]
    """
    (embed_w, embed_b, gru_wi, gru_wh, gru_bh,
     tWq, tWk, tWv, tWo, ln1_s, ln1_b,
     ffw1, ffb1, ffw2, ffb2, ln2_s, ln2_b,
     hw1_1, hb1_1, hw2_1, hb2_1,
     hw1_2, hb1_2, hw2_2, hb2_2,
     hw1_3, hb1_3, hw2_3, hb2_3,
     hw1_4, hb1_4, hw2_4, hb2_4) = params

    obs = full_obs[:, :, :OBS_D]
    cap = full_obs[:, :, OBS_D:]
    emb = jax.nn.relu(obs @ embed_w + embed_b)  # [T, BC, FC]
    resets = dones.astype(bool)

    def gru_step(h, xt):
        e, d = xt
        h = jnp.where(d[:, None], 0.0, h)
        gi = e @ gru_wi
        gh = h @ gru_wh + gru_bh
        ir, iz, inn = jnp.split(gi, 3, -1)
        hr, hz, hn = jnp.split(gh, 3, -1)
        r = jax.nn.sigmoid(ir + hr)
        z = jax.nn.sigmoid(iz + hz)
        n = jnp.tanh(inn + r * hn)
        hnew = (1.0 - z) * n + z * h
        return hnew, hnew

    hidden, emb_seq = jax.lax.scan(gru_step, hidden, (emb, resets))

    # Transformer over the agents dim, for all T*BC samples of this shard.
    n_samp = T * BC
    x = cap.reshape(n_samp, N_AGENTS, N_CAP)
    flag = jnp.zeros((n_samp, N_AGENTS, 1), jnp.float32).at[:, 0, 0].set(1.0)
    x = jnp.concatenate([x, flag], axis=-1)  # [N, A, D]
    hd = D // NH
    scale = 1.0 / jnp.sqrt(jnp.float32(hd))
    for l in range(NL):
        q = (x @ tWq[l]).reshape(-1, N_AGENTS, NH, hd)
        k = (x @ tWk[l]).reshape(-1, N_AGENTS, NH, hd)
        v = (x @ tWv[l]).reshape(-1, N_AGENTS, NH, hd)
        att = jax.nn.softmax(jnp.einsum("nqhd,nkhd->nhqk", q, k) * scale, axis=-1)
        o = jnp.einsum("nhqk,nkhd->nqhd", att, v).reshape(-1, N_AGENTS, D) @ tWo[l]
        x = 2.0 * _ln(o + x, ln1_s[l], ln1_b[l])
        ff = jax.nn.relu(x @ ffw1[l] + ffb1[l]) @ ffw2[l] + ffb2[l]
        x = 2.0 * _ln(ff + x, ln2_s[l], ln2_b[l])
    cap_repr = jnp.mean(x, axis=1).reshape(T, BC, D)

    w1 = _hyper(cap_repr, hw1_1, hb1_1, hw2_1, hb2_1).reshape(T, BC, FC, TGT)
    b1 = _hyper(cap_repr, hw1_2, hb1_2, hw2_2, hb2_2).reshape(T, BC, 1, TGT)
    w2 = _hyper(cap_repr, hw1_3, hb1_3, hw2_3, hb2_3).reshape(T, BC, TGT, A_DIM)
    b2 = _hyper(cap_repr, hw1_4, hb1_4, hw2_4, hb2_4).reshape(T, BC, 1, A_DIM)

    h1 = jax.nn.relu(emb_seq[:, :, None, :] @ w1 + b1)
    logits = (h1 @ w2 + b2).squeeze(2)  # [T, BC, A_DIM]
    return hidden, logits


_pmapped = None
_param_cache = {}


def _get_pmapped():
    global _pmapped
    if _pmapped is None:
        in_axes = (0, 0, 0) + (None,) * len(_PARAM_NAMES)
        _pmapped = jax.pmap(
            _shard_fn,
            in_axes=in_axes,
            devices=jax.devices()[:N_CORES],
        )
    return _pmapped


def _cached_params(inputs):
    """Device-transfer each param once per distinct host array (repeat calls
    with the same numpy objects skip the host->device copy)."""
    out = []
    for k in _PARAM_NAMES:
        arr = inputs[k]
        key = (k, id(arr))
        dev = _param_cache.get(key)
        if dev is None:
            dev = jnp.asarray(arr, jnp.float32)
            _param_cache.clear() if len(_param_cache) > 256 else None
            _param_cache[key] = dev
        out.append(dev)
    return tuple(out)


def kernel(**inputs):
    # Keep full fp32 matmul precision on the PE (default would downcast to
    # bf16-ish precision; the hypernet contractions need f32 accuracy).
    with jax.default_matmul_precision("highest"):
        params = _cached_params(inputs)

        # Shard over B on host (cheap views/copies): core c takes
        # b in [c*BC, (c+1)*BC).
        hidden_np = np.asarray(inputs["hidden"], np.float32)
        obs_np = np.asarray(inputs["full_obs"], np.float32)
        dones_np = np.asarray(inputs["dones"], np.int32)
        hidden_sh = hidden_np.reshape(N_CORES, BC, FC)
        obs_sh = np.ascontiguousarray(
            obs_np.reshape(T, N_CORES, BC, OBS_D + CAP_D).transpose(1, 0, 2, 3)
        )
        dones_sh = np.ascontiguousarray(
            dones_np.reshape(T, N_CORES, BC).transpose(1, 0, 2)
        )

        fn = _get_pmapped()
        hidden_out, logits_out = fn(hidden_sh, obs_sh, dones_sh, *params)

        # Gather / unshard to full shapes.
        hidden_full = np.asarray(hidden_out).reshape(B, FC).astype(np.float32)
        logits_full = (
            np.asarray(logits_out).transpose(1, 0, 2, 3).reshape(T, B, A_DIM)
        ).astype(np.float32)
    return hidden_full, logits_full


if __name__ == "__main__":
    # Smoke test with random inputs of the right shapes.
    rng = np.random.default_rng(0)
    inputs = dict(
        hidden=np.zeros((B, FC), np.float32),
        full_obs=rng.standard_normal((T, B, OBS_D + CAP_D), dtype=np.float32),
        dones=rng.integers(0, 2, (T, B)).astype(np.int32),
        embed_w=rng.standard_normal((OBS_D, FC), dtype=np.float32) * 0.1,
        embed_b=np.zeros((FC,), np.float32),
        gru_wi=rng.standard_normal((FC, 3 * FC), dtype=np.float32) * 0.1,
        gru_wh=rng.standard_normal((FC, 3 * FC), dtype=np.float32) * 0.1,
        gru_bh=np.zeros((3 * FC,), np.float32),
        tWq=rng.standard_normal((NL, D, D), dtype=np.float32) * 0.3,
        tWk=rng.standard_normal((NL, D, D), dtype=np.float32) * 0.3,
        tWv=rng.standard_normal((NL, D, D), dtype=np.float32) * 0.3,
        tWo=rng.standard_normal((NL, D, D), dtype=np.float32) * 0.3,
        ln1_s=np.ones((NL, D), np.float32), ln1_b=np.zeros((NL, D), np.float32),
        ffw1=rng.standard_normal((NL, D, FF), dtype=np.float32) * 0.2,
        ffb1=np.zeros((NL, FF), np.float32),
        ffw2=rng.standard_normal((NL, FF, D), dtype=np.float32) * 0.2,
        ffb2=np.zeros((NL, D), np.float32),
        ln2_s=np.ones((NL, D), np.float32), ln2_b=np.zeros((NL, D), np.float32),
        hw1_1=rng.standard_normal((D, HN), dtype=np.float32) * 0.1,
        hb1_1=np.zeros((HN,), np.float32),
        hw2_1=rng.standard_normal((HN, FC * TGT), dtype=np.float32) * 0.02,
        hb2_1=np.zeros((FC * TGT,), np.float32),
        hw1_2=rng.standard_normal((D, HN), dtype=np.float32) * 0.1,
        hb1_2=np.zeros((HN,), np.float32),
        hw2_2=rng.standard_normal((HN, TGT), dtype=np.float32) * 0.02,
        hb2_2=np.zeros((TGT,), np.float32),
        hw1_3=rng.standard_normal((D, HN), dtype=np.float32) * 0.1,
        hb1_3=np.zeros((HN,), np.float32),
        hw2_3=rng.standard_normal((HN, TGT * A_DIM), dtype=np.float32) * 0.01,
        hb2_3=np.zeros((TGT * A_DIM,), np.float32),
        hw1_4=rng.standard_normal((D, HN), dtype=np.float32) * 0.1,
        hb1_4=np.zeros((HN,), np.float32),
        hw2_4=rng.standard_normal((HN, A_DIM), dtype=np.float32) * 0.01,
        hb2_4=np.zeros((A_DIM,), np.float32),
    )
    h, lg = kernel(**inputs)
    print("hidden:", h.shape, h.dtype, "logits:", lg.shape, lg.dtype)
